# revision 14
# baseline (speedup 1.0000x reference)
"""DiceCE-with-ignore-index loss kernel for Trainium2, 8 NeuronCores.

Contract: kernel(logits, target) -> np.float32 scalar loss, matching
reference: CE (mean over valid voxels) + masked soft Dice (batch dice,
background excluded), ignore_index = -1.

Strategy (v4 — engine-balanced, bit-trick transcendentals, host masks)
----------------------------------------------------------------------
Data-parallel over (b, d): 16 d-slices per core, 1,048,576 voxels each,
processed as NCH=4 chunks of [128, 2048] bf16 planes.  The {0,1} mask
planes eq_c = (t==c) and vf = (t>=0) are exact functions of the int32
target and are materialized host-side during sharding (the target
itself is not shipped); all floating-point math over the logits runs
on device.  Work per engine:

  ACT    e_c = exp(x_c) for c=1..3 ; L = ln s           (4 planes)
  Pool   s01 = e0+e1 ; s23 = e2+e3                      (2 planes)
  DVE    e_0 via Schraudolph bit trick (int16 TS on x0) ;
         s = s01+s23 ; r = 1/s via int16 bit trick
         (r = bitcast(K - i(s)), K=32497) ;
         w = vf*r (TT) ; z_c = e_c*w (TT) ;
         ca-STT accum -> sum vf*ln s
  PE     diag-trick PSUM accumulation, 5 matmuls per 128-col set:
         pk_c  = eq_c  x [z_c | x_c]  -> intersect[c], xb[c]  (c=1..3)
         ps4   = w     x [e1|e2|e3]   -> p_sum[c]
         ps0   = eq_0  x [x_0]        -> xb[0]

e_0 feeds only the softmax denominator (class 0 is excluded from dice
and its xb term uses raw x_0), so the ~2% sawtooth of the bit-trick exp
averages out; calibrated constants keep the aggregate bias ~6e-4.
gt_sum[c] / count are exact host-side integer stats of the target;
intersect/xb/p_sum/ca partials are combined on host in float64.
CE = (sum vf*L - sum_c xb_c) / count.
"""
import os
import sys
from contextlib import ExitStack

for _p in ("/opt/trn_rl_repo", "/root/.axon_site/_ro/trn_rl_repo", "/root/.axon_site"):
    if os.path.isdir(_p) and _p not in sys.path:
        sys.path.append(_p)

import numpy as np
import ml_dtypes

import concourse.bass as bass
import concourse.tile as tile
from concourse import bacc, mybir
from concourse.bass_utils import run_bass_kernel_spmd

BF16 = mybir.dt.bfloat16
I16 = mybir.dt.int16
F32 = mybir.dt.float32
ALU = mybir.AluOpType
ACTF = mybir.ActivationFunctionType

P = 128
FD = 1024        # free dim per chunk
NCH = 8          # chunks per core (8 * 128 * 1024 = 1,048,576 voxels)
NSET = FD // P   # 16 diag sets per chunk
NCORES = 8
C = 4

B, D, H, W = 2, 64, 256, 256
SMOOTH_NR = 1e-05
SMOOTH_DR = 1e-05
RECIP_K = 32497.0        # bf16 bit-trick reciprocal constant (calibrated)
EXP_A = 184.6649652      # 128/ln2
EXP_B = 16248.75         # 128*(127 - mu), mu centers the sawtooth

_NC_CACHE = {}


def _patch_act_tables():
    """Force Exp and Ln into the combined natural_log_exp set: one
    ACT_TABLE_LOAD for the whole kernel."""
    import concourse.hw_specs as hw_specs
    if getattr(bacc, "_act_tables_patched", False):
        return
    orig = hw_specs.get_activation_tables

    def patched(arch):
        tables = {k: set(v) for k, v in orig(arch).items()}
        if "natural_log_exp_and_others" in tables:
            for name, fns in tables.items():
                if name != "natural_log_exp_and_others":
                    fns.discard(ACTF.Exp)
                    fns.discard(ACTF.Ln)
        return tables

    hw_specs.get_activation_tables = patched
    bacc.get_activation_tables = patched
    bacc._act_tables_patched = True


def _build_nc():
    _patch_act_tables()
    nc = bacc.Bacc("TRN2", target_bir_lowering=False, debug=False)

    X = nc.dram_tensor("x", [C, NCH, P, FD], BF16, kind="ExternalInput")
    M = nc.dram_tensor("m", [5, NCH, P, FD], BF16, kind="ExternalInput")  # eq0..eq3, vf
    OUT_ACC = nc.dram_tensor("out_acc", [P, NCH], F32, kind="ExternalOutput")
    # psum blocks: pk1|pk2|pk3 (256 each) | ps4 (384) | ps0 (128)
    OUT_PS = nc.dram_tensor("out_ps", [P, 3 * 256 + 384 + 128], F32, kind="ExternalOutput")

    with tile.TileContext(nc) as tc, ExitStack() as ctx:
        io = ctx.enter_context(tc.tile_pool(name="io", bufs=3))
        wk = ctx.enter_context(tc.tile_pool(name="wk", bufs=3))
        one = ctx.enter_context(tc.tile_pool(name="one", bufs=1))
        psum = ctx.enter_context(tc.tile_pool(name="psum", bufs=1, space="PSUM"))

        acc = one.tile([P, NCH], F32, name="acc")
        pk = [psum.tile([P, 256], F32, name=f"pk{c}") for c in (1, 2, 3)]
        ps4 = psum.tile([P, 384], F32, name="ps4")
        ps0 = psum.tile([P, 128], F32, name="ps0")

        for k in range(NCH):
            first, last = k == 0, k == NCH - 1
            # ---- inputs: one strided DMA for all x planes (into plane 1
            # of the [z|x] slots of ZX), one for all five mask planes ----
            ZX = io.tile([P, C, 2, FD], BF16, tag="ZX", name=f"ZX_{k}")
            EQ = io.tile([P, 5, FD], BF16, tag="EQ", name=f"EQ_{k}")
            nc.sync.dma_start(ZX[:, :, 1, :], X[:, k].transpose([1, 0, 2]))
            nc.sync.dma_start(EQ[:], M[:, k].transpose([1, 0, 2]))
            x0 = ZX[:, 0, 1, :]
            zx = [ZX[:, c, :, :] for c in (1, 2, 3)]
            eq = [EQ[:, c, :] for c in range(C)]
            vf = EQ[:, 4, :]

            # ---- exponentials: e0 bit trick (DVE), e1..3 on ACT ----
            e0I = wk.tile([P, FD], I16, tag="e0I", name=f"e0I_{k}")
            nc.vector.tensor_scalar(out=e0I[:], in0=x0, scalar1=EXP_A,
                                    scalar2=EXP_B, op0=ALU.mult, op1=ALU.add)
            E = wk.tile([P, 3, FD], BF16, tag="E", name=f"E_{k}")
            for i in range(3):
                nc.scalar.activation(E[:, i, :], zx[i][:, 1, :], ACTF.Exp)

            # ---- softmax denominator: Pool partial, DVE the rest ----
            s01 = wk.tile([P, FD], BF16, tag="s01", name=f"s01_{k}")
            s23 = wk.tile([P, FD], BF16, tag="s23", name=f"s23_{k}")
            s = wk.tile([P, FD], BF16, tag="s", name=f"s_{k}")
            nc.gpsimd.tensor_tensor(s01[:], E[:, 0, :], E[:, 1, :], ALU.add)
            nc.vector.tensor_tensor(s23[:], e0I[:].bitcast(BF16), E[:, 2, :], ALU.add)
            nc.vector.tensor_tensor(s[:], s01[:], s23[:], ALU.add)

            # ---- L = ln s (ACT);  r = 1/s via int16 bit trick (DVE) ----
            L = wk.tile([P, FD], BF16, tag="L", name=f"L_{k}")
            nc.scalar.activation(L[:], s[:], ACTF.Ln)
            rI = wk.tile([P, FD], I16, tag="rI", name=f"rI_{k}")
            nc.vector.tensor_scalar(out=rI[:], in0=s[:].bitcast(I16),
                                    scalar1=-1.0, scalar2=RECIP_K,
                                    op0=ALU.mult, op1=ALU.add)

            # ---- w = vf * r ; z_c = e_c * w ----
            w = wk.tile([P, FD], BF16, tag="w", name=f"w_{k}")
            nc.vector.tensor_tensor(w[:], vf, rI[:].bitcast(BF16), ALU.mult)
            for i in range(3):
                nc.vector.tensor_tensor(zx[i][:, 0, :], E[:, i, :], w[:], ALU.mult)

            # ---- ca = sum vf * L (STT with accum; is_ge(vf,0.5) == vf) ----
            scr = wk.tile([P, FD], BF16, tag="scr", name=f"scr_{k}")
            nc.vector.scalar_tensor_tensor(
                out=scr[:], in0=vf, scalar=0.5, in1=L[:],
                op0=ALU.is_ge, op1=ALU.mult,
                accum_out=acc[:, k:k + 1])

            # ---- PE diag-trick accumulation ----
            for j in range(NSET):
                sl = slice(j * P, (j + 1) * P)
                st = (first and j == 0)
                sp = (last and j == NSET - 1)
                nc.tensor.matmul(ps0[:], eq[0][:, sl], x0[:, sl], start=st, stop=sp)
                nc.tensor.matmul(ps4[:], w[:, sl], E[:, :, sl], start=st, stop=sp)
                for i in range(3):
                    nc.tensor.matmul(pk[i][:], eq[i + 1][:, sl], zx[i][:, :, sl],
                                     start=st, stop=sp)

        # ---- epilogue ----
        ps_sb = one.tile([P, 3 * 256 + 384 + 128], F32, name="ps_sb")
        for i in range(3):
            nc.vector.tensor_copy(ps_sb[:, i * 256:(i + 1) * 256], pk[i][:])
        nc.vector.tensor_copy(ps_sb[:, 768:1152], ps4[:])
        nc.vector.tensor_copy(ps_sb[:, 1152:1280], ps0[:])
        nc.sync.dma_start(OUT_ACC[:], acc[:])
        nc.sync.dma_start(OUT_PS[:], ps_sb[:])

    nc.compile()
    return nc


def _get_nc():
    if "nc" not in _NC_CACHE:
        _NC_CACHE["nc"] = _build_nc()
    return _NC_CACHE["nc"]


def _shard_inputs(logits: np.ndarray, target: np.ndarray):
    """Split into 8 per-core input maps; cast logits to bf16 and build
    the {0,1} mask planes host-side.  Also stashes the exact target
    statistics (gt_sum, valid count) used by _combine."""
    assert logits.shape == (B, C, D, H, W), logits.shape
    assert target.shape == (B, 1, D, H, W), target.shape
    lg = np.ascontiguousarray(logits).astype(ml_dtypes.bfloat16)
    t32 = target[:, 0]

    _NC_CACHE["gt"] = np.array([(t32 == c).sum() for c in (1, 2, 3)], np.float64)
    _NC_CACHE["count"] = float((t32 >= 0).sum())

    masks = np.empty((5,) + t32.shape, dtype=ml_dtypes.bfloat16)
    for c in range(C):
        masks[c] = (t32 == c).astype(ml_dtypes.bfloat16)
    masks[4] = (t32 >= 0).astype(ml_dtypes.bfloat16)

    d_per_core = D // (NCORES // B)  # 16
    in_maps = []
    for k in range(NCORES):
        b = k // (NCORES // B)
        d0 = (k % (NCORES // B)) * d_per_core
        xs = lg[b, :, d0:d0 + d_per_core].reshape(C, NCH, P, FD)
        ms = masks[:, b, d0:d0 + d_per_core].reshape(5, NCH, P, FD)
        in_maps.append({"x": np.ascontiguousarray(xs), "m": np.ascontiguousarray(ms)})
    return in_maps


def _combine(results) -> np.float32:
    inter = np.zeros(3, np.float64)
    xb = np.zeros(C, np.float64)
    ps_sum = np.zeros(3, np.float64)
    ca = 0.0

    for res in results:
        ca += res["out_acc"].astype(np.float64).sum()
        blk = res["out_ps"].astype(np.float64)
        for i in range(3):
            b0 = i * 256
            inter[i] += np.trace(blk[:, b0:b0 + 128])
            xb[i + 1] += np.trace(blk[:, b0 + 128:b0 + 256])
            ps_sum[i] += np.trace(blk[:, 768 + i * 128:768 + (i + 1) * 128])
        xb[0] += np.trace(blk[:, 1152:1280])

    gt = _NC_CACHE["gt"]
    count = _NC_CACHE["count"]
    ce = (ca - xb.sum()) / count

    denom = ps_sum + gt
    dice = (2.0 * inter + SMOOTH_NR) / (denom + SMOOTH_DR)
    present = (gt > 0).astype(np.float64)
    n_present = present.sum()
    mean_dice = (dice * present).sum() / max(n_present, 1.0)
    dice_loss = (1.0 - mean_dice) if n_present > 0 else 0.0
    return np.float32(dice_loss + ce)


def kernel(logits: np.ndarray, target: np.ndarray) -> np.ndarray:
    nc = _get_nc()
    in_maps = _shard_inputs(np.asarray(logits), np.asarray(target))
    last_exc = None
    for _attempt in range(3):
        try:
            out = run_bass_kernel_spmd(nc, in_maps, core_ids=list(range(NCORES)))
            return _combine(out.results)
        except Exception as exc:  # transient NRT_EXEC_UNIT_UNRECOVERABLE recovers on retry
            last_exc = exc
            import time
            time.sleep(2.0)
    raise last_exc


if __name__ == "__main__":
    rng = np.random.default_rng(0)
    lg = rng.standard_normal((B, C, D, H, W), dtype=np.float32)
    tg = rng.integers(-1, C, (B, 1, D, H, W)).astype(np.int32)
    print(kernel(lg, tg))


# revision 16
# speedup vs baseline: 1.1536x; 1.1536x over previous
"""DiceCE-with-ignore-index loss kernel for Trainium2, 8 NeuronCores.

Contract: kernel(logits, target) -> np.float32 scalar loss, matching
reference: CE (mean over valid voxels) + masked soft Dice (batch dice,
background excluded), ignore_index = -1.

Strategy (v4 — engine-balanced, bit-trick transcendentals, host masks)
----------------------------------------------------------------------
Data-parallel over (b, d): 16 d-slices per core, 1,048,576 voxels each,
processed as NCH=4 chunks of [128, 2048] bf16 planes.  The {0,1} mask
planes eq_c = (t==c) and vf = (t>=0) are exact functions of the int32
target and are materialized host-side during sharding (the target
itself is not shipped); all floating-point math over the logits runs
on device.  Work per engine:

  ACT    e_c = exp(x_c) for c=1..3 ; L = ln s           (4 planes)
  Pool   s01 = e0+e1 ; s23 = e2+e3                      (2 planes)
  DVE    e_0 via Schraudolph bit trick (int16 TS on x0) ;
         s = s01+s23 ; r = 1/s via int16 bit trick
         (r = bitcast(K - i(s)), K=32497) ;
         w = vf*r (TT) ; z_c = e_c*w (TT) ;
         ca-STT accum -> sum vf*ln s
  PE     diag-trick PSUM accumulation, 5 matmuls per 128-col set:
         pk_c  = eq_c  x [z_c | x_c]  -> intersect[c], xb[c]  (c=1..3)
         ps4   = w     x [e1|e2|e3]   -> p_sum[c]
         ps0   = eq_0  x [x_0]        -> xb[0]

e_0 feeds only the softmax denominator (class 0 is excluded from dice
and its xb term uses raw x_0), so the ~2% sawtooth of the bit-trick exp
averages out; calibrated constants keep the aggregate bias ~6e-4.
gt_sum[c] / count are exact host-side integer stats of the target;
intersect/xb/p_sum/ca partials are combined on host in float64.
CE = (sum vf*L - sum_c xb_c) / count.
"""
import os
import sys
from contextlib import ExitStack

for _p in ("/opt/trn_rl_repo", "/root/.axon_site/_ro/trn_rl_repo", "/root/.axon_site"):
    if os.path.isdir(_p) and _p not in sys.path:
        sys.path.append(_p)

import numpy as np
import ml_dtypes

import concourse.bass as bass
import concourse.tile as tile
from concourse import bacc, mybir
from concourse.bass_utils import run_bass_kernel_spmd

BF16 = mybir.dt.bfloat16
I16 = mybir.dt.int16
F32 = mybir.dt.float32
ALU = mybir.AluOpType
ACTF = mybir.ActivationFunctionType

P = 128
FD = 1024        # free dim per chunk
NCH = 8          # chunks per core (8 * 128 * 1024 = 1,048,576 voxels)
NSET = FD // P   # 16 diag sets per chunk
NCORES = 8
C = 4

B, D, H, W = 2, 64, 256, 256
SMOOTH_NR = 1e-05
SMOOTH_DR = 1e-05
RECIP_K = 32497.0        # bf16 bit-trick reciprocal constant (calibrated)
EXP_A = 184.6649652      # 128/ln2
EXP_B = 16248.75         # 128*(127 - mu), mu centers the sawtooth

_NC_CACHE = {}


def _patch_act_tables():
    """Force Exp and Ln into the combined natural_log_exp set: one
    ACT_TABLE_LOAD for the whole kernel."""
    import concourse.hw_specs as hw_specs
    if getattr(bacc, "_act_tables_patched", False):
        return
    orig = hw_specs.get_activation_tables

    def patched(arch):
        tables = {k: set(v) for k, v in orig(arch).items()}
        if "natural_log_exp_and_others" in tables:
            for name, fns in tables.items():
                if name != "natural_log_exp_and_others":
                    fns.discard(ACTF.Exp)
                    fns.discard(ACTF.Ln)
        return tables

    hw_specs.get_activation_tables = patched
    bacc.get_activation_tables = patched
    bacc._act_tables_patched = True


def _build_nc():
    _patch_act_tables()
    nc = bacc.Bacc("TRN2", target_bir_lowering=False, debug=False)

    X = nc.dram_tensor("x", [C, NCH, P, FD], BF16, kind="ExternalInput")
    M = nc.dram_tensor("m", [5, NCH, P, FD], BF16, kind="ExternalInput")  # eq0..eq3, vf
    OUT_ACC = nc.dram_tensor("out_acc", [P, NCH], F32, kind="ExternalOutput")
    # psum blocks: pk1|pk2|pk3 (256 each) | ps4 (384) | ps0 (128)
    OUT_PS = nc.dram_tensor("out_ps", [P, 3 * 256 + 384 + 128], F32, kind="ExternalOutput")

    with tile.TileContext(nc) as tc, ExitStack() as ctx:
        io = ctx.enter_context(tc.tile_pool(name="io", bufs=3))
        wk = ctx.enter_context(tc.tile_pool(name="wk", bufs=3))
        one = ctx.enter_context(tc.tile_pool(name="one", bufs=1))
        psum = ctx.enter_context(tc.tile_pool(name="psum", bufs=1, space="PSUM"))

        acc = one.tile([P, NCH], F32, name="acc")
        pk = [psum.tile([P, 256], F32, name=f"pk{c}") for c in (1, 2, 3)]
        ps4 = psum.tile([P, 384], F32, name="ps4")
        ps0 = psum.tile([P, 128], F32, name="ps0")

        for k in range(NCH):
            first, last = k == 0, k == NCH - 1
            # ---- inputs: x1..x3 land in plane 1 of the [z|x] pack tiles ----
            x0 = io.tile([P, FD], BF16, tag="x0", name=f"x0_{k}")
            zx = [io.tile([P, 2, FD], BF16, tag=f"zx{c}", name=f"zx{c}_{k}")
                  for c in (1, 2, 3)]
            eq = [io.tile([P, FD], BF16, tag=f"eq{c}", name=f"eq{c}_{k}")
                  for c in range(C)]
            vf = io.tile([P, FD], BF16, tag="vf", name=f"vf_{k}")
            nc.sync.dma_start(x0[:], X[0, k])
            for i in range(3):
                nc.sync.dma_start(zx[i][:, 1, :], X[i + 1, k])
            for c in range(C):
                nc.sync.dma_start(eq[c][:], M[c, k])
            nc.sync.dma_start(vf[:], M[4, k])

            # ---- exponentials: e0 bit trick (DVE), e1..3 on ACT ----
            e0I = wk.tile([P, FD], I16, tag="e0I", name=f"e0I_{k}")
            nc.vector.tensor_scalar(out=e0I[:], in0=x0[:], scalar1=EXP_A,
                                    scalar2=EXP_B, op0=ALU.mult, op1=ALU.add)
            E = wk.tile([P, 3, FD], BF16, tag="E", name=f"E_{k}")
            for i in range(3):
                nc.scalar.activation(E[:, i, :], zx[i][:, 1, :], ACTF.Exp)

            # ---- softmax denominator: Pool partial, DVE the rest ----
            s01 = wk.tile([P, FD], BF16, tag="s01", name=f"s01_{k}")
            s23 = wk.tile([P, FD], BF16, tag="s23", name=f"s23_{k}")
            s = wk.tile([P, FD], BF16, tag="s", name=f"s_{k}")
            nc.vector.tensor_tensor(s01[:], E[:, 0, :], E[:, 1, :], ALU.add)
            nc.vector.tensor_tensor(s23[:], e0I[:].bitcast(BF16), E[:, 2, :], ALU.add)
            nc.vector.tensor_tensor(s[:], s01[:], s23[:], ALU.add)

            # ---- L = ln s (ACT);  r = 1/s via int16 bit trick (DVE) ----
            L = wk.tile([P, FD], BF16, tag="L", name=f"L_{k}")
            nc.scalar.activation(L[:], s[:], ACTF.Ln)
            rI = wk.tile([P, FD], I16, tag="rI", name=f"rI_{k}")
            nc.vector.tensor_scalar(out=rI[:], in0=s[:].bitcast(I16),
                                    scalar1=-1.0, scalar2=RECIP_K,
                                    op0=ALU.mult, op1=ALU.add)

            # ---- w = vf * r ; z_c = e_c * w ----
            w = wk.tile([P, FD], BF16, tag="w", name=f"w_{k}")
            nc.vector.tensor_tensor(w[:], vf[:], rI[:].bitcast(BF16), ALU.mult)
            for i in range(3):
                nc.vector.tensor_tensor(zx[i][:, 0, :], E[:, i, :], w[:], ALU.mult)

            # ---- ca = sum vf * L (STT with accum; is_ge(vf,0.5) == vf) ----
            scr = wk.tile([P, FD], BF16, tag="scr", name=f"scr_{k}")
            nc.vector.scalar_tensor_tensor(
                out=scr[:], in0=vf[:], scalar=0.5, in1=L[:],
                op0=ALU.is_ge, op1=ALU.mult,
                accum_out=acc[:, k:k + 1])

            # ---- PE diag-trick accumulation ----
            for j in range(NSET):
                sl = slice(j * P, (j + 1) * P)
                st = (first and j == 0)
                sp = (last and j == NSET - 1)
                nc.tensor.matmul(ps0[:], eq[0][:, sl], x0[:, sl], start=st, stop=sp)
                nc.tensor.matmul(ps4[:], w[:, sl], E[:, :, sl], start=st, stop=sp)
                for i in range(3):
                    nc.tensor.matmul(pk[i][:], eq[i + 1][:, sl], zx[i][:, :, sl],
                                     start=st, stop=sp)

        # ---- epilogue ----
        ps_sb = one.tile([P, 3 * 256 + 384 + 128], F32, name="ps_sb")
        for i in range(3):
            nc.vector.tensor_copy(ps_sb[:, i * 256:(i + 1) * 256], pk[i][:])
        nc.vector.tensor_copy(ps_sb[:, 768:1152], ps4[:])
        nc.vector.tensor_copy(ps_sb[:, 1152:1280], ps0[:])
        nc.sync.dma_start(OUT_ACC[:], acc[:])
        nc.sync.dma_start(OUT_PS[:], ps_sb[:])

    nc.compile()
    return nc


def _get_nc():
    if "nc" not in _NC_CACHE:
        _NC_CACHE["nc"] = _build_nc()
    return _NC_CACHE["nc"]


def _shard_inputs(logits: np.ndarray, target: np.ndarray):
    """Split into 8 per-core input maps; cast logits to bf16 and build
    the {0,1} mask planes host-side.  Also stashes the exact target
    statistics (gt_sum, valid count) used by _combine."""
    assert logits.shape == (B, C, D, H, W), logits.shape
    assert target.shape == (B, 1, D, H, W), target.shape
    lg = np.ascontiguousarray(logits).astype(ml_dtypes.bfloat16)
    t32 = target[:, 0]

    _NC_CACHE["gt"] = np.array([(t32 == c).sum() for c in (1, 2, 3)], np.float64)
    _NC_CACHE["count"] = float((t32 >= 0).sum())

    masks = np.empty((5,) + t32.shape, dtype=ml_dtypes.bfloat16)
    for c in range(C):
        masks[c] = (t32 == c).astype(ml_dtypes.bfloat16)
    masks[4] = (t32 >= 0).astype(ml_dtypes.bfloat16)

    d_per_core = D // (NCORES // B)  # 16
    in_maps = []
    for k in range(NCORES):
        b = k // (NCORES // B)
        d0 = (k % (NCORES // B)) * d_per_core
        xs = lg[b, :, d0:d0 + d_per_core].reshape(C, NCH, P, FD)
        ms = masks[:, b, d0:d0 + d_per_core].reshape(5, NCH, P, FD)
        in_maps.append({"x": np.ascontiguousarray(xs), "m": np.ascontiguousarray(ms)})
    return in_maps


def _combine(results) -> np.float32:
    inter = np.zeros(3, np.float64)
    xb = np.zeros(C, np.float64)
    ps_sum = np.zeros(3, np.float64)
    ca = 0.0

    for res in results:
        ca += res["out_acc"].astype(np.float64).sum()
        blk = res["out_ps"].astype(np.float64)
        for i in range(3):
            b0 = i * 256
            inter[i] += np.trace(blk[:, b0:b0 + 128])
            xb[i + 1] += np.trace(blk[:, b0 + 128:b0 + 256])
            ps_sum[i] += np.trace(blk[:, 768 + i * 128:768 + (i + 1) * 128])
        xb[0] += np.trace(blk[:, 1152:1280])

    gt = _NC_CACHE["gt"]
    count = _NC_CACHE["count"]
    ce = (ca - xb.sum()) / count

    denom = ps_sum + gt
    dice = (2.0 * inter + SMOOTH_NR) / (denom + SMOOTH_DR)
    present = (gt > 0).astype(np.float64)
    n_present = present.sum()
    mean_dice = (dice * present).sum() / max(n_present, 1.0)
    dice_loss = (1.0 - mean_dice) if n_present > 0 else 0.0
    return np.float32(dice_loss + ce)


def kernel(logits: np.ndarray, target: np.ndarray) -> np.ndarray:
    nc = _get_nc()
    in_maps = _shard_inputs(np.asarray(logits), np.asarray(target))
    last_exc = None
    for _attempt in range(3):
        try:
            out = run_bass_kernel_spmd(nc, in_maps, core_ids=list(range(NCORES)))
            return _combine(out.results)
        except Exception as exc:  # transient NRT_EXEC_UNIT_UNRECOVERABLE recovers on retry
            last_exc = exc
            import time
            time.sleep(2.0)
    raise last_exc


if __name__ == "__main__":
    rng = np.random.default_rng(0)
    lg = rng.standard_normal((B, C, D, H, W), dtype=np.float32)
    tg = rng.integers(-1, C, (B, 1, D, H, W)).astype(np.int32)
    print(kernel(lg, tg))
